# revision 44
# baseline (speedup 1.0000x reference)
"""Trainium2 Bass kernel for AttentionConvFull (local 5x5 window attention
with per-channel softmax, grouped 1x1 conv projections).

Sharding: 8 cores = batch(4) x H-halves(2). Each core gets a 32-row halo'd,
zero-padded slice of x, pre-transposed on host to channel-major [256, 32*60].
No collectives needed.

v6 dataflow (per core, 2 channel chunks of 128 partitions, chunk-serial
j-loops sharing the 4 PSUM accumulator buffers with the projection tiles):
  PE    : block-diag 128x128 bf16 matmuls for q/k/v projections; per window
          offset j, identity-matmul PSUM accumulation of den += e_j and
          num += w_j (streams at full 2.4 GHz when fed back-to-back).
  DVE   : kr = k_win + rel_j (tensor_scalar 4x) for ~60% of j's; t = kr*qf
          (flat*flat 2x); w = e*v_win (2x); epilogue recip+mult;
          v-projection copies.
  ACT   : e = exp(t) batched over j-pairs (FD=3136 amortizes the fixed
          activate cost); the other ~40% of kr adds (Identity+bias); k/q
          projection copies (q fused with +q_emb bias); flat-qf extraction.
  DMA   : 1-element-shifted copies of the k/v maps so odd window columns
          keep 4B alignment for the DVE packed modes.
GPSIMD is deliberately unused in the j-loop: its SBUF port is shared with
the DVE, and concurrent Q7 tensor ops were measured to stall 2-port DVE
instructions by 2-5x.
Software-pipelined emission: pair i's kr/t/exp are emitted before pair
i-1's w-mults and PE accumulations, so the in-order DVE queue never waits
on ACT's exp of the same pair.
Epilogue: out = num * recip_approx(den); DMA out channel-major; host
reassembles to (B,H,W,C).
"""

import numpy as np
import ml_dtypes

import concourse.bass as bass
import concourse.tile as tile
from concourse import bacc, mybir
from concourse.bass_utils import run_bass_kernel_spmd

F32 = mybir.dt.float32
BF16 = mybir.dt.bfloat16

K = 5
G = 8
B, H, W, C = 4, 56, 56, 256
Cg = C // G            # 32
P = K // 2             # 2
HS = H // 2            # 28 output rows per shard
MR = HS + 2 * P        # 32 map rows
MC = W + 2 * P         # 60 map cols
SP = MR * MC           # 1920 map spatial
OP = HS * W            # 1568 output spatial per shard
NCH = 2                # channel chunks of 128 partitions
NCORES = 8
HALF = OP // 2         # 784: PSUM accumulate tile half-size
NS = 2                 # projection psum splits
SL = SP // NS          # 960 cols per projection block

# jj positions (dj-major order, per chunk) whose kr = k + rel_j add runs
# on ACT (Identity+bias, ~1.9us) instead of DVE tensor_scalar (~0.62us),
# tuned so both engines finish together
ACT_KR_JJ = {0, 2, 4, 5, 7, 9, 11, 12, 14, 16, 17, 19, 21, 23}


def _dedup_ldweights(nc):
    """Remove redundant PE weight reloads: consecutive InstLdweights that
    load the same stationary operand with no sync info. The identity matrix
    stays resident across the whole accumulation loop, so only the first
    load is needed. Self-loading matmuls (bf16 projections) reset the
    tracked weight state."""
    removed = 0
    for blk in nc.main_func.blocks:
        last_sig = None
        keep = []
        for inst in blk.instructions:
            if isinstance(inst, mybir.InstLdweights):
                sig = " ".join(a.concise() for a in inst.ins)
                si = inst.sync_info
                clean = si is None or (
                    len(si.on_wait) == 0 and len(si.on_update) == 0
                )
                if sig == last_sig and clean:
                    removed += 1
                    continue
                last_sig = sig
            elif isinstance(inst, mybir.InstMatmult):
                if len(inst.ins) > 1:
                    wsig = inst.ins[1].concise()
                    if wsig != last_sig:
                        last_sig = wsig
            keep.append(inst)
        blk.instructions[:] = keep
    return removed


def build_nc():
    nc = bacc.Bacc(
        "TRN2", target_bir_lowering=False, debug=False, num_devices=NCORES
    )

    xt_d = nc.dram_tensor("xt", [NCH, 128, SP], BF16, kind="ExternalInput").ap()
    wq_d = nc.dram_tensor("wqb", [NCH, 128, 128], BF16, kind="ExternalInput").ap()
    wk_d = nc.dram_tensor("wkb", [NCH, 128, 128], BF16, kind="ExternalInput").ap()
    wv_d = nc.dram_tensor("wvb", [NCH, 128, 128], BF16, kind="ExternalInput").ap()
    rel_d = nc.dram_tensor("relb", [NCH, 128, K * K], F32, kind="ExternalInput").ap()
    qe_d = nc.dram_tensor("qeb", [NCH, 128, 1], F32, kind="ExternalInput").ap()
    id_d = nc.dram_tensor("idn", [128, 128], BF16, kind="ExternalInput").ap()
    out_d = nc.dram_tensor("out", [NCH, 128, OP], F32, kind="ExternalOutput").ap()

    with tile.TileContext(nc) as tc:
        with (
            tc.tile_pool(name="consts", bufs=1) as consts,
            tc.tile_pool(name="weights", bufs=2) as wpool,
            tc.tile_pool(name="xin", bufs=2) as xpool,
            tc.tile_pool(name="maps", bufs=1) as mpool,
            tc.tile_pool(name="jwork", bufs=4) as jpool,
            tc.tile_pool(name="wwork", bufs=4) as wwpool,
            tc.tile_pool(name="epi", bufs=2) as epool,
            tc.tile_pool(name="acc", bufs=1, space=bass.MemorySpace.PSUM) as psum,
        ):
            # ident is not needed until the first den accumulation ~15us
            # in; the scalar queue issues it without delaying the x loads
            ident = consts.tile([128, 128], BF16, tag="ident")
            nc.scalar.dma_start(ident[:], id_d)

            # whole-kernel maps: both chunks side by side
            k_all = mpool.tile([128, NCH * SP], BF16, tag="k")
            v_all = mpool.tile([128, NCH * SP], BF16, tag="v")
            ko_all = mpool.tile([128, NCH * SP], BF16, tag="ko")
            vo_all = mpool.tile([128, NCH * SP], BF16, tag="vo")
            qf_all = mpool.tile([128, NCH * OP], BF16, tag="qf")

            rels = []

            # ---- projections ----
            for c in range(NCH):
                # chunk 1's input DMAs go out on the (otherwise idle)
                # gpsimd queue so both chunks' loads run in parallel
                dq = nc.sync if c == 0 else nc.gpsimd
                x_sb = xpool.tile([128, SP], BF16, tag="x", name=f"x{c}")
                wts = {}
                for nm in ("wk", "wq", "wv"):
                    wts[nm] = wpool.tile(
                        [128, 128], BF16, tag=nm, name=f"{nm}{c}"
                    )
                # issue order = need order: x block 0, wk, wq, then the rest
                dq.dma_start(x_sb[:, :SL], xt_d[c][:, :SL])
                dq.dma_start(wts["wk"][:], wk_d[c])
                dq.dma_start(wts["wq"][:], wq_d[c])
                dq.dma_start(x_sb[:, SL:], xt_d[c][:, SL:])
                dq.dma_start(wts["wv"][:], wv_d[c])
                rel_sb = wpool.tile([128, K * K], F32, tag="rel", name=f"rel{c}")
                dq.dma_start(rel_sb[:], rel_d[c])
                qe_sb = wpool.tile([128, 1], F32, tag="qe", name=f"qe{c}")
                dq.dma_start(qe_sb[:], qe_d[c])
                rels.append(rel_sb)

                co = c * SP
                rot = 0
                for nm, dstmap in (("wk", k_all), ("wq", None), ("wv", v_all)):
                    for s in range(NS):
                        lo = s * SL
                        # projection psum tiles share the 4 accumulator
                        # buffers (tags a0-a3); WAR deps serialize correctly
                        ps = psum.tile(
                            [128, SL], F32, tag=f"a{rot % 4}", name=f"pp{c}{s}{nm}"
                        )
                        rot += 1
                        for mlo, mn in ((0, 512), (512, SL - 512)):
                            nc.tensor.matmul(
                                ps[:, mlo : mlo + mn],
                                wts[nm][:],
                                x_sb[:, lo + mlo : lo + mlo + mn],
                                start=True,
                                stop=True,
                            )
                        if nm == "wq":
                            # interior rows/cols of this 16-map-row band
                            # straight into flat qf, with the q_emb bias
                            r0 = max(P, 16 * s)
                            r1 = min(MR - P, 16 * (s + 1))
                            src = ps[:].rearrange("p (h w) -> p h w", h=16)[
                                :, r0 - 16 * s : r1 - 16 * s, P : P + W
                            ]
                            dst = qf_all[:, c * OP : (c + 1) * OP].rearrange(
                                "p (h w) -> p h w", h=HS
                            )[:, r0 - P : r1 - P, :]
                            nc.scalar.activation(
                                dst,
                                src,
                                mybir.ActivationFunctionType.Identity,
                                bias=qe_sb[:],
                            )
                        else:
                            # k/v copies ride the DVE, which is otherwise
                            # idle during the projection phase
                            dstmap = k_all if nm == "wk" else v_all
                            nc.vector.tensor_copy(
                                dstmap[:, co + lo : co + lo + SL], ps[:]
                            )

                # 1-elem-shifted copies (contiguous SBUF->SBUF DMA, cheap):
                # x_od[i] = x[i+1], so odd-dj window reads stay 4B-aligned
                nc.sync.dma_start(
                    ko_all[:, co : co + SP - 1], k_all[:, co + 1 : co + SP]
                )
                nc.sync.dma_start(
                    vo_all[:, co : co + SP - 1], v_all[:, co + 1 : co + SP]
                )

            k4 = k_all[:].rearrange("p (c h w) -> p c h w", c=NCH, h=MR)
            ko4 = ko_all[:].rearrange("p (c h w) -> p c h w", c=NCH, h=MR)
            v4 = v_all[:].rearrange("p (c h w) -> p c h w", c=NCH, h=MR)
            vo4 = vo_all[:].rearrange("p (c h w) -> p c h w", c=NCH, h=MR)

            # ---- j-loop per chunk (chunk-serial: PSUM holds one chunk) ----
            # dj-major order: the odd-shifted maps (needed from dj=1)
            # arrive via DMA while the dj=0 iterations run
            JORD = [di * K + dj for dj in range(K) for di in range(K)]
            pending_epi = None
            for c in range(NCH):
                rel_sb = rels[c]
                qf = qf_all[:, c * OP : (c + 1) * OP]

                den = [
                    psum.tile([128, HALF], F32, tag=f"a{h}", name=f"den{c}{h}")
                    for h in range(2)
                ]
                num = [
                    psum.tile([128, HALF], F32, tag=f"a{2 + h}", name=f"num{c}{h}")
                    for h in range(2)
                ]

                def win_views(jj):
                    j = JORD[jj]
                    di, dj = j // K, j % K
                    if dj % 2 == 0:
                        ks, vs, dje = k4, v4, dj
                    else:
                        ks, vs, dje = ko4, vo4, dj - 1
                    kv = ks[:, c, di : di + HS, dje : dje + W]
                    vv = vs[:, c, di : di + HS, dje : dje + W]
                    return j, kv, vv

                PAIRS = [
                    (jj0, jj0 + 1) if jj0 + 1 < K * K else (jj0,)
                    for jj0 in range(0, K * K, 2)
                ]

                def emit_t(pi):
                    jjs = PAIRS[pi]
                    npr = len(jjs)
                    t_pair = jpool.tile(
                        [128, 2 * OP], BF16, tag="t", name=f"t{c}p{pi}"
                    )
                    e_pair = jpool.tile(
                        [128, 2 * OP], BF16, tag="e", name=f"e{c}p{pi}"
                    )
                    kr_p = wwpool.tile(
                        [128, 2 * OP], BF16, tag="kr", name=f"kr{c}p{pi}"
                    )
                    vvs = []
                    for m, jj in enumerate(jjs):
                        j, kv, vv = win_views(jj)
                        vvs.append(vv)
                        kr3 = kr_p[:, m * OP : (m + 1) * OP].rearrange(
                            "p (h w) -> p h w", h=HS
                        )
                        if jj in ACT_KR_JJ:
                            nc.scalar.activation(
                                kr3,
                                kv,
                                mybir.ActivationFunctionType.Identity,
                                bias=rel_sb[:, j : j + 1],
                            )
                        else:
                            nc.vector.tensor_scalar(
                                kr3,
                                kv,
                                rel_sb[:, j : j + 1],
                                None,
                                mybir.AluOpType.add,
                            )
                    # one mult covers the pair: qf broadcast over the
                    # pair dim (stride-0 outer)
                    n = npr * OP
                    qf_b = qf.unsqueeze(1).broadcast_to([128, npr, OP])
                    nc.vector.tensor_tensor(
                        t_pair[:, :n].rearrange("p (m f) -> p m f", m=npr),
                        kr_p[:, :n].rearrange("p (m f) -> p m f", m=npr),
                        qf_b,
                        mybir.AluOpType.mult,
                    )
                    nc.scalar.activation(
                        e_pair[:, :n],
                        t_pair[:, :n],
                        mybir.ActivationFunctionType.Exp,
                    )
                    return jjs, e_pair, vvs

                def emit_wmm(state):
                    jjs, e_pair, vvs = state
                    w_ts = []
                    for m, jj in enumerate(jjs):
                        w_t = wwpool.tile(
                            [128, OP], BF16, tag="w", name=f"w{c}{jj}"
                        )
                        w3 = w_t[:].rearrange("p (h w) -> p h w", h=HS)
                        e3 = e_pair[:, m * OP : (m + 1) * OP].rearrange(
                            "p (h w) -> p h w", h=HS
                        )
                        nc.vector.tensor_tensor(
                            w3, e3, vvs[m], mybir.AluOpType.mult
                        )
                        w_ts.append(w_t)
                    # all den matmuls (one wait on e_pair), then the nums
                    for m, jj in enumerate(jjs):
                        st = jj == 0
                        sp = jj == K * K - 1
                        for h in range(2):
                            base = m * OP + h * HALF
                            for lo, n in ((0, 512), (512, HALF - 512)):
                                nc.tensor.matmul(
                                    den[h][:, lo : lo + n],
                                    ident[:],
                                    e_pair[:, base + lo : base + lo + n],
                                    start=st,
                                    stop=sp,
                                )
                    for m, jj in enumerate(jjs):
                        st = jj == 0
                        sp = jj == K * K - 1
                        for h in range(2):
                            base = h * HALF
                            for lo, n in ((0, 512), (512, HALF - 512)):
                                nc.tensor.matmul(
                                    num[h][:, lo : lo + n],
                                    ident[:],
                                    w_ts[m][:, base + lo : base + lo + n],
                                    start=st,
                                    stop=sp,
                                )

                prev = None
                for pi in range(len(PAIRS)):
                    state = emit_t(pi)
                    # the previous chunk's epilogue is deferred into this
                    # chunk's pipeline so ACT keeps streaming exps across
                    # the chunk boundary while DVE drains the PSUM reads
                    if pi == 1 and pending_epi is not None:
                        pending_epi()
                        pending_epi = None
                    if prev is not None:
                        emit_wmm(prev)
                    prev = state
                emit_wmm(prev)

                def make_epilogue(c, den, num):
                    # out = num * recip(den), per half; DMA overlaps
                    def emit():
                        out_sb = epool.tile(
                            [128, OP], F32, tag="osb", name=f"osb{c}"
                        )
                        for h in range(2):
                            base = h * HALF
                            rden = epool.tile(
                                [128, HALF], F32, tag="rden", name=f"rd{c}{h}"
                            )
                            nc.vector.reciprocal_approx_fast(rden[:], den[h][:])
                            nc.vector.tensor_tensor(
                                out_sb[:, base : base + HALF],
                                num[h][:],
                                rden[:],
                                mybir.AluOpType.mult,
                            )
                            nc.sync.dma_start(
                                out_d[c][:, base : base + HALF],
                                out_sb[:, base : base + HALF],
                            )
                    return emit

                pending_epi = make_epilogue(c, den, num)
            pending_epi()

    nc.compile()
    _dedup_ldweights(nc)
    return nc


def _block_diag_weights(w):
    """w: (G, Cg_out, Cg_in) -> lhsT layout [NCH, 128, 128] where
    lhsT[c, ci, co] = w[g, co%32, ci%32] for matching 32-blocks."""
    out = np.zeros((NCH, 128, 128), np.float32)
    for c in range(NCH):
        for g4 in range(4):
            g = c * 4 + g4
            blk = w[g]  # (Cg_out, Cg_in)
            out[c, g4 * 32 : (g4 + 1) * 32, g4 * 32 : (g4 + 1) * 32] = blk.T
    return out


_NC_CACHE = {}


def _make_in_maps(inputs):
    x = np.asarray(inputs["x"], np.float32)
    wq = np.asarray(inputs["wq"], np.float32)
    wk = np.asarray(inputs["wk"], np.float32)
    wv = np.asarray(inputs["wv"], np.float32)
    rel_emb = np.asarray(inputs["rel_emb"], np.float32)
    q_emb = np.asarray(inputs["q_emb"], np.float32)

    bf = ml_dtypes.bfloat16
    wqb = _block_diag_weights(wq).astype(bf)
    wkb = _block_diag_weights(wk).astype(bf)
    wvb = _block_diag_weights(wv).astype(bf)
    relb = np.ascontiguousarray(
        rel_emb.reshape(G, Cg, K * K).reshape(NCH, 128, K * K)
    )
    qeb = np.ascontiguousarray(q_emb.reshape(NCH, 128, 1))
    idn = np.eye(128, dtype=bf)

    xp = np.pad(x, ((0, 0), (P, P), (P, P), (0, 0)))  # (B, 60, 60, C)

    in_maps = []
    for core in range(NCORES):
        b, half = divmod(core, 2)
        sh = xp[b, HS * half : HS * half + MR]         # (32, 60, C)
        xt = np.ascontiguousarray(sh.reshape(SP, C).T).reshape(NCH, 128, SP)
        in_maps.append(
            {
                "xt": xt.astype(bf),
                "wqb": wqb,
                "wkb": wkb,
                "wvb": wvb,
                "relb": relb,
                "qeb": qeb,
                "idn": idn,
            }
        )
    return in_maps


def kernel(**inputs):
    in_maps = _make_in_maps(inputs)

    if "nc" not in _NC_CACHE:
        _NC_CACHE["nc"] = build_nc()
    nc = _NC_CACHE["nc"]

    res = run_bass_kernel_spmd(nc, in_maps, core_ids=list(range(NCORES)))

    out = np.empty((B, H, W, C), np.float32)
    for core in range(NCORES):
        b, half = divmod(core, 2)
        o = res.results[core]["out"].reshape(C, HS, W)
        out[b, HS * half : HS * half + HS] = o.transpose(1, 2, 0)
    return out
